# revision 1
# baseline (speedup 1.0000x reference)
"""Trainium2 Bass kernel for nn_CrossAttention (B=8, Sq=Skv=2048, D=1024, C=768).

Strategy: data-parallel over batch — each of the 8 NeuronCores computes one
batch element's full cross-attention.

Per-core pipeline (all matmuls in float32r — TF32-like, 4x faster than fp32):
  phase 1a: K^T = Wk @ ctx^T (+bk) staged to DRAM;  V = ctx @ Wv^T (+bv) kept
            resident in SBUF [k, d].
  phase 1b: Q^T = (Wq @ x^T + bq)/sqrt(D) staged to DRAM in [d, q] layout.
  phase 2 (per 512-wide q block):
      scores^T[k,q] = KT_tile.T @ QT  (accumulate over d)        -> PSUM
      expT = exp(scores^T)            (ACT evacuation, no max — scores are
                                       bounded: |s| < ~3 for this problem)
      sums[1,q]  += ones.T @ expT     (PE matmul per k-tile)
      out^T[d,q]  = V_slice.T @ expT  (accumulate over k)
      final[q,o]  = outT_slice.T @ WoT (accumulate over d)
      final evac: * (1/sums[q]) (per-partition ACT scale) + bo, DMA out.

Softmax normalization commutes with the (linear) out-projection, so 1/sum is
applied on the final tiles where q sits on partitions.
"""

import numpy as np

import concourse.bass as bass  # noqa: F401  (bass types used via bacc/tile)
import concourse.mybir as mybir
import concourse.tile as tile
from concourse import bacc
from concourse.bass_utils import run_bass_kernel_spmd

# ---- problem shapes (hardcoded) ----
B, SQ, SKV, D, C = 8, 2048, 2048, 1024, 768
P = 128
DT = D // P          # 8  d-tiles
CT = C // P          # 6  c-tiles
KT = SKV // P        # 16 k-tiles
QB = 512             # q block width
NQB = SQ // QB       # 4 q blocks
KC = 512             # k chunk width in phase 1a
NKC = SKV // KC      # 4
SCALE = 1.0 / np.sqrt(np.float32(D))

F32 = mybir.dt.float32
F32R = mybir.dt.float32r
AF = mybir.ActivationFunctionType

_NC_CACHE = {}


def build():
    if "nc" in _NC_CACHE:
        return _NC_CACHE["nc"]
    nc = bacc.Bacc(trn_type="TRN2", num_swdge_queues=4)

    # ---- DRAM I/O (per-core slices; names = in_map keys) ----
    xT = nc.dram_tensor("xT", [D, SQ], F32R, kind="ExternalInput")
    ctxT = nc.dram_tensor("ctxT", [C, SKV], F32R, kind="ExternalInput")
    WqT = nc.dram_tensor("WqT", [D, D], F32R, kind="ExternalInput")
    WkT = nc.dram_tensor("WkT", [C, D], F32R, kind="ExternalInput")
    WvT = nc.dram_tensor("WvT", [C, D], F32R, kind="ExternalInput")
    WoT = nc.dram_tensor("WoT", [D, D], F32R, kind="ExternalInput")
    bqh = nc.dram_tensor("bqh", [P, DT], F32, kind="ExternalInput")  # bq*scale, [p, dt]
    bkh = nc.dram_tensor("bkh", [P, DT], F32, kind="ExternalInput")
    bvb = nc.dram_tensor("bvb", [P, D], F32, kind="ExternalInput")   # bv broadcast
    bob = nc.dram_tensor("bob", [P, D], F32, kind="ExternalInput")   # bo broadcast
    onesmat = nc.dram_tensor("onesmat", [P, P], F32R, kind="ExternalInput")  # all 1.0
    e0two = nc.dram_tensor("e0two", [P, 2], F32R, kind="ExternalInput")  # row0=1 else 0
    out = nc.dram_tensor("out", [SQ, D], F32, kind="ExternalOutput")

    with tile.TileContext(nc) as tc:
        with tc.tile_pool(name="persist", bufs=1) as persist, \
             tc.tile_pool(name="dstage", bufs=1, space="DRAM") as dstage:
            # intermediate stagings (DRAM pool tiles so Tile tracks the
            # staging-write -> reload-read dependency; raw dram_tensors are
            # not dep-tracked and the reloads would race the writes)
            KTst = dstage.tile([KT, DT, P, P], F32R, name="KTst")
            # one staging tile per q-block so phase 2's block-0 reload only
            # depends on block-0's writes (not all of phase 1b)
            QTst = [dstage.tile([DT, P, QB], F32R, name=f"QTst{qb}")
                    for qb in range(NQB)]
            v_sb = persist.tile([P, KT, D], F32R, name="v_sb")          # 64KB/p
            bq_sb = persist.tile([P, DT], F32, name="bq_sb")
            bk_sb = persist.tile([P, DT], F32, name="bk_sb")
            bv_sb = persist.tile([P, D], F32, name="bv_sb")
            bo_sb = persist.tile([P, D], F32, name="bo_sb")
            om_sb = persist.tile([P, P], F32R, name="om_sb")
            e0_sb = persist.tile([P, 2], F32R, name="e0_sb")
            sums_sb = persist.tile([P, QB], F32R, name="sums_sb")
            nc.sync.dma_start(bq_sb, bqh[:])
            nc.sync.dma_start(bk_sb, bkh[:])
            nc.sync.dma_start(bv_sb, bvb[:])
            nc.sync.dma_start(bo_sb, bob[:])
            nc.sync.dma_start(om_sb, onesmat[:])
            nc.sync.dma_start(e0_sb, e0two[:])

            # ================= phase 1a: K^T staging + V resident =========
            with tc.tile_pool(name="p1a_w", bufs=1) as p1a_w, \
                 tc.tile_pool(name="p1a_s", bufs=2) as p1a_s, \
                 tc.tile_pool(name="p1a_stg", bufs=4) as p1a_stg, \
                 tc.tile_pool(name="ps_k", bufs=2, space="PSUM") as ps_k, \
                 tc.tile_pool(name="ps_v", bufs=2, space="PSUM") as ps_v:
                wk_sb = p1a_w.tile([P, CT, D], F32R, name="wk_sb")
                wv_sb = p1a_w.tile([P, CT, D], F32R, name="wv_sb")
                # ~128KB DMA chunks (per-queue BW is only ~22GB/s) issued in
                # need-order so the first matmul group's operands land first
                ctx_tiles = []
                for kc in range(NKC):
                    ctx_tiles.append(
                        p1a_s.tile([P, CT, KC], F32R, name="ctx_sb", tag="ctx")
                        if kc < 2 else None)
                for t in range(CT):
                    nc.sync.dma_start(
                        ctx_tiles[0][:, t],
                        ctxT[t * P:(t + 1) * P, 0:KC])
                for quarter in range(4):
                    for t in range(CT):
                        nc.sync.dma_start(
                            wk_sb[:, t, quarter * 256:(quarter + 1) * 256],
                            WkT[t * P:(t + 1) * P, quarter * 256:(quarter + 1) * 256])
                for quarter in range(4):
                    for t in range(CT):
                        nc.sync.dma_start(
                            wv_sb[:, t, quarter * 256:(quarter + 1) * 256],
                            WvT[t * P:(t + 1) * P, quarter * 256:(quarter + 1) * 256])
                for t in range(CT):
                    nc.sync.dma_start(ctx_tiles[1][:, t],
                                      ctxT[t * P:(t + 1) * P, KC:2 * KC])
                for kc in range(NKC):
                    if ctx_tiles[kc] is None:
                        ctx_tiles[kc] = p1a_s.tile([P, CT, KC], F32R,
                                                   name="ctx_sb", tag="ctx")
                    ctx_sb = ctx_tiles[kc]
                    if kc > 1:
                        for t in range(CT):
                            nc.sync.dma_start(
                                ctx_sb[:, t],
                                ctxT[t * P:(t + 1) * P, kc * KC:(kc + 1) * KC])
                    # K^T tiles [d=128, k=512] for each d-tile
                    for dt_ in range(DT):
                        pk = ps_k.tile([P, KC], F32, name="pk", tag="pk")
                        for ct_ in range(CT):
                            nc.tensor.matmul(
                                pk, wk_sb[:, ct_, dt_ * P:(dt_ + 1) * P],
                                ctx_sb[:, ct_, :],
                                start=(ct_ == 0), stop=(ct_ == CT - 1))
                        kstg = p1a_stg.tile([P, KC], F32R, name="kstg", tag="kstg")
                        nc.scalar.activation(kstg, pk, AF.Identity,
                                             bias=bk_sb[:, dt_:dt_ + 1])
                        nc.gpsimd.dma_start(
                            KTst[4 * kc:4 * kc + 4, dt_].rearrange(
                                "t p i -> p t i"),
                            kstg.rearrange("p (t i) -> p t i", t=4))
                    # V tiles [k=128, d] resident
                    for t in range(4):
                        kt_ = kc * 4 + t
                        for dh in range(2):
                            pv = ps_v.tile([P, 512], F32, name="pv", tag="pv")
                            for ct_ in range(CT):
                                nc.tensor.matmul(
                                    pv, ctx_sb[:, ct_, t * P:(t + 1) * P],
                                    wv_sb[:, ct_, dh * 512:(dh + 1) * 512],
                                    start=(ct_ == 0), stop=(ct_ == CT - 1))
                            nc.vector.tensor_add(
                                v_sb[:, kt_, dh * 512:(dh + 1) * 512],
                                pv, bv_sb[:, dh * 512:(dh + 1) * 512])

            # wo pool spans 1b+2; its loads are issued mid-1b (after qb0's
            # gating loads) so phase 2 never waits on it
            with tc.tile_pool(name="p2_w", bufs=1) as p2_w, \
                 tc.tile_pool(name="p2_qt", bufs=2) as p2_qt:
                wo_sb = p2_w.tile([P, DT, D], F32R, name="wo_sb")
                qt0_sb = None

                # ================= phase 1b: Q^T staging ==================
                with tc.tile_pool(name="p1b_w", bufs=1) as p1b_w, \
                     tc.tile_pool(name="p1b_s", bufs=2) as p1b_s, \
                     tc.tile_pool(name="p1b_stg", bufs=2) as p1b_stg, \
                     tc.tile_pool(name="ps_q", bufs=2, space="PSUM") as ps_q:
                    wq_sb = p1b_w.tile([P, DT, D], F32R, name="wq_sb")
                    for t in range(DT):
                        nc.sync.dma_start(wq_sb[:, t, 0:256],
                                          WqT[t * P:(t + 1) * P, 0:256])
                    for qb in range(NQB):
                        xt_sb = p1b_s.tile([P, DT, QB], F32R, name="xt_sb",
                                           tag="xt")
                        for t in range(DT):
                            nc.sync.dma_start(
                                xt_sb[:, t],
                                xT[t * P:(t + 1) * P, qb * QB:(qb + 1) * QB])
                        if qb == 0:
                            for quarter in range(1, 4):
                                for t in range(DT):
                                    nc.sync.dma_start(
                                        wq_sb[:, t,
                                              quarter * 256:(quarter + 1) * 256],
                                        WqT[t * P:(t + 1) * P,
                                            quarter * 256:(quarter + 1) * 256])
                        if qb == 1:
                            # prefetch phase-2 block-0 QT (QTst[0] was
                            # finished during qb==0)
                            qt0_sb = p2_qt.tile([P, DT, QB], F32R,
                                                name="qt_sb", tag="qt")
                            for t in range(DT):
                                nc.sync.dma_start(qt0_sb[:, t], QTst[0][t])
                        for dt_ in range(DT):
                            pq = ps_q.tile([P, QB], F32, name="pq", tag="pq")
                            for it in range(DT):
                                nc.tensor.matmul(
                                    pq, wq_sb[:, it, dt_ * P:(dt_ + 1) * P],
                                    xt_sb[:, it, :],
                                    start=(it == 0), stop=(it == DT - 1))
                            qstg = p1b_stg.tile([P, QB], F32R, name="qstg",
                                                tag="qstg")
                            nc.scalar.activation(qstg, pq, AF.Identity,
                                                 bias=bq_sb[:, dt_:dt_ + 1],
                                                 scale=float(SCALE))
                            nc.gpsimd.dma_start(QTst[qb][dt_], qstg)

                # ============== phase 2: attention + out proj =============
                with tc.tile_pool(name="p2_kts", bufs=3) as p2_kts, \
                     tc.tile_pool(name="p2_big", bufs=1) as p2_big, \
                     tc.tile_pool(name="p2_fin", bufs=4) as p2_fin, \
                     tc.tile_pool(name="p2_rcp", bufs=2) as p2_rcp, \
                     tc.tile_pool(name="ps_sc", bufs=2, space="PSUM") as ps_sc, \
                     tc.tile_pool(name="ps_sum", bufs=1, space="PSUM") as ps_sum, \
                     tc.tile_pool(name="ps_rt", bufs=1, space="PSUM") as ps_rt, \
                     tc.tile_pool(name="ps_out", bufs=2, space="PSUM") as ps_out, \
                     tc.tile_pool(name="ps_fin", bufs=2, space="PSUM") as ps_fin:
                  for qb in range(NQB):
                    if qb == 0:
                        qt_sb = qt0_sb
                    else:
                        qt_sb = p2_qt.tile([P, DT, QB], F32R, name="qt_sb",
                                           tag="qt")
                        for t in range(DT):
                            nc.sync.dma_start(qt_sb[:, t], QTst[qb][t])
                    expt_sb = p2_big.tile([P, KT, QB], F32R, name="expt_sb",
                                          tag="expt")
                    psums = ps_sum.tile([P, QB], F32, name="psums", tag="psums")
                    # ---- scores^T + exp + denominator ----
                    for kt_ in range(KT):
                        kts = p2_kts.tile([P, DT, P], F32R, name="kts",
                                          tag="kts")
                        nc.sync.dma_start(
                            kts, KTst[kt_].rearrange("d p i -> p d i"))
                        psc = ps_sc.tile([P, QB], F32, name="psc", tag="psc")
                        for dt_ in range(DT):
                            nc.tensor.matmul(
                                psc, kts[:, dt_], qt_sb[:, dt_],
                                start=(dt_ == 0), stop=(dt_ == DT - 1))
                        nc.scalar.activation(expt_sb[:, kt_], psc, AF.Exp)
                        # every output partition gets the k-sum of expT
                        nc.tensor.matmul(
                            psums, om_sb, expt_sb[:, kt_],
                            start=(kt_ == 0), stop=(kt_ == KT - 1),
                            skip_group_check=True)
                    if qb == 0:
                        # wo streams in while qb0's PV runs; needed only by
                        # the final projection ~40us later
                        for quarter in range(4):
                            for t in range(DT):
                                nc.sync.dma_start(
                                    wo_sb[:, t,
                                          quarter * 256:(quarter + 1) * 256],
                                    WoT[t * P:(t + 1) * P,
                                        quarter * 256:(quarter + 1) * 256])
                    # ---- 1/sums, transposed to [q-on-partition, 1] ----
                    nc.scalar.copy(sums_sb, psums)
                    prt = ps_rt.tile([P, 8], F32, name="prt", tag="prt")
                    for qs in range(4):
                        nc.tensor.matmul(
                            prt[:, 2 * qs:2 * qs + 2],
                            sums_sb[:, qs * P:(qs + 1) * P], e0_sb,
                            start=True, stop=True)
                    recip = p2_rcp.tile([P, 8], F32, name="recip", tag="recip")
                    nc.vector.reciprocal(recip, prt)
                    # ---- out^T = V.T @ expT (d-quarter passes) ----
                    outt_sb = p2_big.tile([P, DT, QB], F32R, name="outt_sb",
                                          tag="outt")
                    for dp in range(4):
                        po0 = ps_out.tile([P, QB], F32, name="po0", tag="po")
                        po1 = ps_out.tile([P, QB], F32, name="po1", tag="po")
                        po = (po0, po1)
                        for kt_ in range(KT):
                            for dc in range(2):
                                d0 = dp * 256 + dc * P
                                nc.tensor.matmul(
                                    po[dc], v_sb[:, kt_, d0:d0 + P],
                                    expt_sb[:, kt_],
                                    start=(kt_ == 0), stop=(kt_ == KT - 1))
                        for dc in range(2):
                            nc.scalar.copy(outt_sb[:, dp * 2 + dc], po[dc])
                    # ---- final = out^T.T @ WoT, * 1/sums + bo ----
                    for qs in range(4):
                        for oc in range(2):
                            pf = ps_fin.tile([P, 512], F32, name="pf", tag="pf")
                            for dt_ in range(DT):
                                nc.tensor.matmul(
                                    pf, outt_sb[:, dt_, qs * P:(qs + 1) * P],
                                    wo_sb[:, dt_, oc * 512:(oc + 1) * 512],
                                    start=(dt_ == 0), stop=(dt_ == DT - 1))
                            fin = p2_fin.tile([P, 512], F32, name="fin",
                                              tag="fin")
                            nc.scalar.mul(fin, pf, recip[:, 2 * qs:2 * qs + 1])
                            nc.vector.tensor_add(fin, fin,
                                                 bo_sb[:, oc * 512:(oc + 1) * 512])
                            nc.sync.dma_start(
                                out[qb * QB + qs * P: qb * QB + (qs + 1) * P,
                                    oc * 512:(oc + 1) * 512], fin)
    nc.finalize()
    _NC_CACHE["nc"] = nc
    return nc


def _host_prep(x, context, Wq, bq, Wk, bk, Wv, bv, Wo, bo):
    """Build the 8 per-core input maps (host-side layout prep)."""
    x = np.asarray(x, dtype=np.float32)
    context = np.asarray(context, dtype=np.float32)
    WqT = np.ascontiguousarray(np.asarray(Wq, np.float32).T)   # [i, d]
    WkT = np.ascontiguousarray(np.asarray(Wk, np.float32).T)   # [c, d]
    WvT = np.ascontiguousarray(np.asarray(Wv, np.float32).T)   # [c, d]
    WoT = np.ascontiguousarray(np.asarray(Wo, np.float32).T)   # [d, o]
    scale = np.float32(1.0 / np.sqrt(np.float32(D)))
    bqh = np.ascontiguousarray(
        (np.asarray(bq, np.float32) * scale).reshape(DT, P).T)  # [p, dt]
    bkh = np.ascontiguousarray(np.asarray(bk, np.float32).reshape(DT, P).T)
    bvb = np.ascontiguousarray(
        np.broadcast_to(np.asarray(bv, np.float32)[None, :], (P, D)))
    bob = np.ascontiguousarray(
        np.broadcast_to(np.asarray(bo, np.float32)[None, :], (P, D)))
    onesmat = np.ones((P, P), np.float32)
    e0two = np.zeros((P, 2), np.float32)
    e0two[0, :] = 1.0
    shared = dict(WqT=WqT, WkT=WkT, WvT=WvT, WoT=WoT, bqh=bqh, bkh=bkh,
                  bvb=bvb, bob=bob, onesmat=onesmat, e0two=e0two)
    in_maps = []
    for b in range(B):
        m = dict(shared)
        m["xT"] = np.ascontiguousarray(x[b].T)        # [D, SQ]
        m["ctxT"] = np.ascontiguousarray(context[b].T)  # [C, SKV]
        in_maps.append(m)
    return in_maps


def kernel(**inputs) -> np.ndarray:
    nc = build()
    in_maps = _host_prep(**inputs)
    res = run_bass_kernel_spmd(nc, in_maps, core_ids=list(range(B)))
    return np.stack([res.results[b]["out"] for b in range(B)], axis=0)



# revision 3
# speedup vs baseline: 1.2840x; 1.2840x over previous
"""Trainium2 Bass kernel for nn_CrossAttention (B=8, Sq=Skv=2048, D=1024, C=768).

Strategy: data-parallel over batch — each of the 8 NeuronCores computes one
batch element's full cross-attention.

v2 design (vs v1 which staged K^T/Q^T via DRAM in f32r):
  * all matmul operands in bf16 — same PE throughput as f32r at free>=256
    (1 cycle/row) but half the DMA traffic and half the SBUF footprint.
  * K^T, V, Q^T and Wo live fully SBUF-resident; zero intermediate DRAM
    staging (v1 round-tripped 32 MB through HBM).
  * inputs stream in over TWO hardware DGE queues (sync + scalar) in
    compute-need order as a handful of large rearranged DMAs.
  * PE warmup matmuls at t=0 keep the tensor engine's p-state at full clock
    while the first operands land (idle PE restarts at 1.2 GHz for ~3 us).
  * softmax denominators accumulated across k-tiles on the DVE (16 tensor_adds)
    + one ones-matmul per q-block instead of 16 PE matmuls per q-block.

Per-core pipeline:
  phase 1a: K^T[d,k] = Wk @ ctx^T (+bk) -> SBUF bf16; V[k,d] = ctx @ Wv^T
            (+bv) -> SBUF bf16.  (per 512-wide k chunk)
  phase 1b: Q^T[d,q] = (Wq @ x^T + bq)/sqrt(D) -> SBUF bf16. (per 512 q chunk)
  phase 2 (per 512-wide q block):
      scores^T[k,q] = KT.T @ QT (accum over d)           -> PSUM
      expT = exp(scores^T) (ACT evac, no max: |s| small)  -> SBUF bf16
      partial[p,q] += expT[kt]  on DVE;  sums = ones.T @ partial (1 matmul)
      out^T[d,q] = V.T @ expT (accum over k)
      final[q,o] = outT.T @ WoT (accum over d); * 1/sums + bo; DMA out.

Softmax normalization commutes with the (linear) out-projection, so 1/sum is
applied on the final tiles where q sits on partitions.
"""

import numpy as np
import ml_dtypes

import concourse.bass as bass  # noqa: F401  (bass types used via bacc/tile)
import concourse.mybir as mybir
import concourse.tile as tile
from concourse import bacc
from concourse.bass_utils import run_bass_kernel_spmd

# ---- problem shapes (hardcoded) ----
B, SQ, SKV, D, C = 8, 2048, 2048, 1024, 768
P = 128
DT = D // P          # 8  d-tiles
CT = C // P          # 6  c-tiles
KT = SKV // P        # 16 k-tiles
QB = 512             # q block width
NQB = SQ // QB       # 4 q blocks
KC = 512             # k chunk width in phase 1a
NKC = SKV // KC      # 4
SCALE = 1.0 / np.sqrt(np.float32(D))
WARM_N = 14          # PE warmup matmuls covering the initial DMA window

F32 = mybir.dt.float32
F32R = mybir.dt.float32r
BF = mybir.dt.bfloat16
AF = mybir.ActivationFunctionType

_NC_CACHE = {}


def build():
    if "nc" in _NC_CACHE:
        return _NC_CACHE["nc"]
    nc = bacc.Bacc(trn_type="TRN2", num_swdge_queues=4)

    # ---- DRAM I/O (per-core slices; names = in_map keys) ----
    xT = nc.dram_tensor("xT", [D, SQ], BF, kind="ExternalInput")
    ctxT = nc.dram_tensor("ctxT", [C, SKV], BF, kind="ExternalInput")
    WqT = nc.dram_tensor("WqT", [D, D], BF, kind="ExternalInput")
    WkT = nc.dram_tensor("WkT", [C, D], BF, kind="ExternalInput")
    WvT = nc.dram_tensor("WvT", [C, D], BF, kind="ExternalInput")
    WoT = nc.dram_tensor("WoT", [D, D], BF, kind="ExternalInput")
    bqh = nc.dram_tensor("bqh", [P, DT], F32, kind="ExternalInput")  # bq*scale
    bkh = nc.dram_tensor("bkh", [P, DT], F32, kind="ExternalInput")
    bvb = nc.dram_tensor("bvb", [P, D], F32, kind="ExternalInput")   # bv bcast
    bob = nc.dram_tensor("bob", [P, D], F32, kind="ExternalInput")   # bo bcast
    onesmat = nc.dram_tensor("onesmat", [P, P], F32R, kind="ExternalInput")
    e0two = nc.dram_tensor("e0two", [P, 2], F32R, kind="ExternalInput")
    out = nc.dram_tensor("out", [SQ, D], F32, kind="ExternalOutput")

    with tile.TileContext(nc) as tc:
        with tc.tile_pool(name="persist", bufs=1) as persist:
            kt_sb = persist.tile([P, DT, SKV], BF, name="kt_sb")    # 32KB/p
            v_sb = persist.tile([P, KT, D], BF, name="v_sb")        # 32KB/p
            qt_sb = persist.tile([P, DT, SQ], BF, name="qt_sb")     # 32KB/p
            wo_sb = persist.tile([P, DT, D], BF, name="wo_sb")      # 16KB/p
            bq_sb = persist.tile([P, DT], F32, name="bq_sb")
            bk_sb = persist.tile([P, DT], F32, name="bk_sb")
            bv_sb = persist.tile([P, D], F32, name="bv_sb")
            bo_sb = persist.tile([P, D], F32, name="bo_sb")
            om_sb = persist.tile([P, P], F32R, name="om_sb")
            e0_sb = persist.tile([P, 2], F32R, name="e0_sb")
            warm_sb = persist.tile([P, 512], BF, name="warm_sb")
            sums_sb = persist.tile([P, QB], F32R, name="sums_sb")
            # consts on the scalar HW queue (tiny; sync queue stays free
            # for the gating wk/ctx loads)
            nc.scalar.dma_start(bq_sb, bqh[:])
            nc.scalar.dma_start(bk_sb, bkh[:])
            nc.scalar.dma_start(bv_sb, bvb[:])
            nc.scalar.dma_start(bo_sb, bob[:])
            nc.scalar.dma_start(om_sb, onesmat[:])
            nc.scalar.dma_start(e0_sb, e0two[:])

            # p1b pools opened early so x/wq prefetch DMAs can be issued
            # while phase 1a computes (SBUF: coexists with 1a pools)
            with tc.tile_pool(name="p1b_w", bufs=1) as p1b_w, \
                 tc.tile_pool(name="p1b_s", bufs=2) as p1b_s:
                wq_sb = p1b_w.tile([P, DT, D], BF, name="wq_sb")
                xt_tiles = [None] * NQB

                # ================= phase 1a: K^T + V resident =============
                with tc.tile_pool(name="p1a_w", bufs=1) as p1a_w, \
                     tc.tile_pool(name="p1a_s", bufs=2) as p1a_s, \
                     tc.tile_pool(name="ps_w", bufs=2, space="PSUM") as ps_w, \
                     tc.tile_pool(name="ps_k", bufs=2, space="PSUM") as ps_k, \
                     tc.tile_pool(name="ps_v", bufs=2, space="PSUM") as ps_v:
                    wk_sb = p1a_w.tile([P, CT, D], BF, name="wk_sb")
                    wv_sb = p1a_w.tile([P, CT, D], BF, name="wv_sb")

                    # PE warmup: keep the tensor engine busy (and its
                    # p-state ramping to 2.4 GHz) while the first real
                    # operands stream in.
                    nc.gpsimd.memset(warm_sb[:], 0.0)
                    for _ in range(WARM_N):
                        pw = ps_w.tile([P, 512], F32, name="pw", tag="pw")
                        nc.tensor.matmul(pw, warm_sb[:, 0:P], warm_sb[:],
                                         start=True, stop=True)

                    # sync queue: need-ordered big loads
                    nc.sync.dma_start(
                        wk_sb[:, :, 0:512],
                        WkT[:, 0:512].rearrange("(c p) d -> p c d", p=P))
                    ctx_tiles = [None] * NKC
                    for kc in range(2):
                        ctx_tiles[kc] = p1a_s.tile([P, CT, KC], BF,
                                                   name="ctx_sb", tag="ctx")
                    nc.sync.dma_start(
                        ctx_tiles[0][:],
                        ctxT[:, 0:KC].rearrange("(c p) k -> p c k", p=P))
                    nc.sync.dma_start(
                        wk_sb[:, :, 512:1024],
                        WkT[:, 512:1024].rearrange("(c p) d -> p c d", p=P))
                    nc.sync.dma_start(
                        ctx_tiles[1][:],
                        ctxT[:, KC:2 * KC].rearrange("(c p) k -> p c k", p=P))
                    # scalar queue: wv halves (V needed ~10us in)
                    for h in range(2):
                        nc.scalar.dma_start(
                            wv_sb[:, :, h * 512:(h + 1) * 512],
                            WvT[:, h * 512:(h + 1) * 512].rearrange(
                                "(c p) d -> p c d", p=P))

                    for kc in range(NKC):
                        if ctx_tiles[kc] is None:
                            ctx_tiles[kc] = p1a_s.tile([P, CT, KC], BF,
                                                       name="ctx_sb",
                                                       tag="ctx")
                            nc.sync.dma_start(
                                ctx_tiles[kc][:],
                                ctxT[:, kc * KC:(kc + 1) * KC].rearrange(
                                    "(c p) k -> p c k", p=P))
                        ctx_sb = ctx_tiles[kc]
                        # K^T tiles [d=128, k=512] per d-tile
                        for dt_ in range(DT):
                            pk = ps_k.tile([P, KC], F32, name="pk", tag="pk")
                            for ct_ in range(CT):
                                nc.tensor.matmul(
                                    pk, wk_sb[:, ct_, dt_ * P:(dt_ + 1) * P],
                                    ctx_sb[:, ct_, :],
                                    start=(ct_ == 0), stop=(ct_ == CT - 1))
                            nc.scalar.activation(
                                kt_sb[:, dt_, kc * KC:(kc + 1) * KC], pk,
                                AF.Identity, bias=bk_sb[:, dt_:dt_ + 1])
                        # V tiles [k=128, d] resident
                        for t in range(4):
                            kt_ = kc * 4 + t
                            for dh in range(2):
                                pv = ps_v.tile([P, 512], F32, name="pv",
                                               tag="pv")
                                for ct_ in range(CT):
                                    nc.tensor.matmul(
                                        pv, ctx_sb[:, ct_, t * P:(t + 1) * P],
                                        wv_sb[:, ct_, dh * 512:(dh + 1) * 512],
                                        start=(ct_ == 0), stop=(ct_ == CT - 1))
                                nc.vector.tensor_add(
                                    v_sb[:, kt_, dh * 512:(dh + 1) * 512],
                                    pv, bv_sb[:, dh * 512:(dh + 1) * 512])
                        # prefetches interleaved into the queues mid-1a
                        if kc == 0:
                            nc.scalar.dma_start(
                                wq_sb[:, :, 0:512],
                                WqT[:, 0:512].rearrange(
                                    "(i p) d -> p i d", p=P))
                        if kc == 1:
                            ctx_tiles[2] = p1a_s.tile([P, CT, KC], BF,
                                                      name="ctx_sb",
                                                      tag="ctx")
                            nc.sync.dma_start(
                                ctx_tiles[2][:],
                                ctxT[:, 2 * KC:3 * KC].rearrange(
                                    "(c p) k -> p c k", p=P))
                            nc.scalar.dma_start(
                                wq_sb[:, :, 512:1024],
                                WqT[:, 512:1024].rearrange(
                                    "(i p) d -> p i d", p=P))
                            xt_tiles[0] = p1b_s.tile([P, DT, QB], BF,
                                                     name="xt_sb", tag="xt")
                            nc.sync.dma_start(
                                xt_tiles[0][:],
                                xT[:, 0:QB].rearrange("(i p) q -> p i q", p=P))
                        if kc == 2:
                            ctx_tiles[3] = p1a_s.tile([P, CT, KC], BF,
                                                      name="ctx_sb",
                                                      tag="ctx")
                            nc.sync.dma_start(
                                ctx_tiles[3][:],
                                ctxT[:, 3 * KC:4 * KC].rearrange(
                                    "(c p) k -> p c k", p=P))
                            xt_tiles[1] = p1b_s.tile([P, DT, QB], BF,
                                                     name="xt_sb", tag="xt")
                            nc.sync.dma_start(
                                xt_tiles[1][:],
                                xT[:, QB:2 * QB].rearrange(
                                    "(i p) q -> p i q", p=P))
                            nc.scalar.dma_start(
                                wo_sb[:, :, 0:512],
                                WoT[:, 0:512].rearrange(
                                    "(i p) d -> p i d", p=P))
                        if kc == 3:
                            nc.scalar.dma_start(
                                wo_sb[:, :, 512:1024],
                                WoT[:, 512:1024].rearrange(
                                    "(i p) d -> p i d", p=P))

                # ================= phase 1b: Q^T resident =================
                with tc.tile_pool(name="ps_q", bufs=2, space="PSUM") as ps_q:
                    for qb in range(NQB):
                        if xt_tiles[qb] is None:
                            xt_tiles[qb] = p1b_s.tile([P, DT, QB], BF,
                                                      name="xt_sb", tag="xt")
                            nc.sync.dma_start(
                                xt_tiles[qb][:],
                                xT[:, qb * QB:(qb + 1) * QB].rearrange(
                                    "(i p) q -> p i q", p=P))
                        xt_sb = xt_tiles[qb]
                        for dt_ in range(DT):
                            pq = ps_q.tile([P, QB], F32, name="pq", tag="pq")
                            for it in range(DT):
                                nc.tensor.matmul(
                                    pq, wq_sb[:, it, dt_ * P:(dt_ + 1) * P],
                                    xt_sb[:, it, :],
                                    start=(it == 0), stop=(it == DT - 1))
                            nc.scalar.activation(
                                qt_sb[:, dt_, qb * QB:(qb + 1) * QB], pq,
                                AF.Identity, bias=bq_sb[:, dt_:dt_ + 1],
                                scale=float(SCALE))

            # ============== phase 2: attention + out proj =============
            with tc.tile_pool(name="p2_big", bufs=1) as p2_big, \
                 tc.tile_pool(name="p2_par", bufs=2) as p2_par, \
                 tc.tile_pool(name="p2_fin", bufs=4) as p2_fin, \
                 tc.tile_pool(name="p2_rcp", bufs=2) as p2_rcp, \
                 tc.tile_pool(name="ps_sc", bufs=2, space="PSUM") as ps_sc, \
                 tc.tile_pool(name="ps_sum", bufs=1, space="PSUM") as ps_sum, \
                 tc.tile_pool(name="ps_rt", bufs=1, space="PSUM") as ps_rt, \
                 tc.tile_pool(name="ps_out", bufs=2, space="PSUM") as ps_out, \
                 tc.tile_pool(name="ps_fin", bufs=2, space="PSUM") as ps_fin:
              for qb in range(NQB):
                expt_sb = p2_big.tile([P, KT, QB], BF, name="expt_sb",
                                      tag="expt")
                partial = p2_par.tile([P, QB], F32R, name="partial",
                                      tag="par")
                # ---- scores^T + exp + DVE partial-sum chain ----
                for kt_ in range(KT):
                    psc = ps_sc.tile([P, QB], F32, name="psc", tag="psc")
                    for dt_ in range(DT):
                        nc.tensor.matmul(
                            psc, kt_sb[:, dt_, kt_ * P:(kt_ + 1) * P],
                            qt_sb[:, dt_, qb * QB:(qb + 1) * QB],
                            start=(dt_ == 0), stop=(dt_ == DT - 1))
                    nc.scalar.activation(expt_sb[:, kt_], psc, AF.Exp)
                    if kt_ == 0:
                        nc.vector.tensor_copy(partial, expt_sb[:, 0])
                    else:
                        nc.vector.tensor_add(partial, partial,
                                             expt_sb[:, kt_])
                # ---- sums over partitions (1 matmul) + 1/sums on q ----
                psums = ps_sum.tile([P, QB], F32, name="psums", tag="psums")
                nc.tensor.matmul(psums, om_sb, partial, start=True, stop=True)
                nc.scalar.copy(sums_sb, psums)
                prt = ps_rt.tile([P, 8], F32, name="prt", tag="prt")
                for qs in range(4):
                    nc.tensor.matmul(
                        prt[:, 2 * qs:2 * qs + 2],
                        sums_sb[:, qs * P:(qs + 1) * P], e0_sb,
                        start=True, stop=True)
                recip = p2_rcp.tile([P, 8], F32, name="recip", tag="recip")
                nc.vector.reciprocal(recip, prt)
                # ---- out^T = V.T @ expT (d-quarter passes) ----
                outt_sb = p2_big.tile([P, DT, QB], BF, name="outt_sb",
                                      tag="outt")
                for dp in range(4):
                    po0 = ps_out.tile([P, QB], F32, name="po0", tag="po")
                    po1 = ps_out.tile([P, QB], F32, name="po1", tag="po")
                    po = (po0, po1)
                    for kt_ in range(KT):
                        for dc in range(2):
                            d0 = dp * 256 + dc * P
                            nc.tensor.matmul(
                                po[dc], v_sb[:, kt_, d0:d0 + P],
                                expt_sb[:, kt_],
                                start=(kt_ == 0), stop=(kt_ == KT - 1))
                    for dc in range(2):
                        nc.scalar.copy(outt_sb[:, dp * 2 + dc], po[dc])
                # ---- final = out^T.T @ WoT, * 1/sums + bo ----
                for qs in range(4):
                    for oc in range(2):
                        pf = ps_fin.tile([P, 512], F32, name="pf", tag="pf")
                        for dt_ in range(DT):
                            nc.tensor.matmul(
                                pf, outt_sb[:, dt_, qs * P:(qs + 1) * P],
                                wo_sb[:, dt_, oc * 512:(oc + 1) * 512],
                                start=(dt_ == 0), stop=(dt_ == DT - 1))
                        fin = p2_fin.tile([P, 512], F32, name="fin",
                                          tag="fin")
                        nc.scalar.mul(fin, pf, recip[:, 2 * qs:2 * qs + 1])
                        nc.vector.tensor_add(
                            fin, fin, bo_sb[:, oc * 512:(oc + 1) * 512])
                        nc.gpsimd.dma_start(
                            out[qb * QB + qs * P: qb * QB + (qs + 1) * P,
                                oc * 512:(oc + 1) * 512], fin)
    nc.finalize()
    _NC_CACHE["nc"] = nc
    return nc


def _host_prep(x, context, Wq, bq, Wk, bk, Wv, bv, Wo, bo):
    """Build the 8 per-core input maps (host-side layout prep)."""
    bf = ml_dtypes.bfloat16
    x = np.asarray(x, dtype=np.float32)
    context = np.asarray(context, dtype=np.float32)
    WqT = np.ascontiguousarray(np.asarray(Wq, np.float32).T).astype(bf)
    WkT = np.ascontiguousarray(np.asarray(Wk, np.float32).T).astype(bf)
    WvT = np.ascontiguousarray(np.asarray(Wv, np.float32).T).astype(bf)
    WoT = np.ascontiguousarray(np.asarray(Wo, np.float32).T).astype(bf)
    scale = np.float32(1.0 / np.sqrt(np.float32(D)))
    bqh = np.ascontiguousarray(
        (np.asarray(bq, np.float32) * scale).reshape(DT, P).T)  # [p, dt]
    bkh = np.ascontiguousarray(np.asarray(bk, np.float32).reshape(DT, P).T)
    bvb = np.ascontiguousarray(
        np.broadcast_to(np.asarray(bv, np.float32)[None, :], (P, D)))
    bob = np.ascontiguousarray(
        np.broadcast_to(np.asarray(bo, np.float32)[None, :], (P, D)))
    onesmat = np.ones((P, P), np.float32)
    e0two = np.zeros((P, 2), np.float32)
    e0two[0, :] = 1.0
    shared = dict(WqT=WqT, WkT=WkT, WvT=WvT, WoT=WoT, bqh=bqh, bkh=bkh,
                  bvb=bvb, bob=bob, onesmat=onesmat, e0two=e0two)
    in_maps = []
    for b in range(B):
        m = dict(shared)
        m["xT"] = np.ascontiguousarray(x[b].T).astype(bf)        # [D, SQ]
        m["ctxT"] = np.ascontiguousarray(context[b].T).astype(bf)  # [C, SKV]
        in_maps.append(m)
    return in_maps


def kernel(**inputs) -> np.ndarray:
    nc = build()
    in_maps = _host_prep(**inputs)
    res = run_bass_kernel_spmd(nc, in_maps, core_ids=list(range(B)))
    return np.stack([res.results[b]["out"] for b in range(B)], axis=0)


# revision 10
# speedup vs baseline: 1.2966x; 1.0098x over previous
"""Trainium2 Bass kernel for nn_CrossAttention (B=8, Sq=Skv=2048, D=1024, C=768).

Strategy: data-parallel over batch — each of the 8 NeuronCores computes one
batch element's full cross-attention.

v2 design (vs v1 which staged K^T/Q^T via DRAM in f32r):
  * all matmul operands in bf16 — same PE throughput as f32r at free>=256
    (1 cycle/row) but half the DMA traffic and half the SBUF footprint.
  * K^T, V, Q^T and Wo live fully SBUF-resident; zero intermediate DRAM
    staging (v1 round-tripped 32 MB through HBM).
  * inputs stream in over TWO hardware DGE queues (sync + scalar) in
    compute-need order as a handful of large rearranged DMAs.
  * PE warmup matmuls at t=0 keep the tensor engine's p-state at full clock
    while the first operands land (idle PE restarts at 1.2 GHz for ~3 us).
  * softmax denominators accumulated across k-tiles on the DVE (16 tensor_adds)
    + one ones-matmul per q-block instead of 16 PE matmuls per q-block.

Per-core pipeline:
  phase 1a: K^T[d,k] = Wk @ ctx^T (+bk) -> SBUF bf16; V[k,d] = ctx @ Wv^T
            (+bv) -> SBUF bf16.  (per 512-wide k chunk)
  phase 1b: Q^T[d,q] = (Wq @ x^T + bq)/sqrt(D) -> SBUF bf16. (per 512 q chunk)
  phase 2 (per 512-wide q block):
      scores^T[k,q] = KT.T @ QT (accum over d)           -> PSUM
      expT = exp(scores^T) (ACT evac, no max: |s| small)  -> SBUF bf16
      partial[p,q] += expT[kt]  on DVE;  sums = ones.T @ partial (1 matmul)
      out^T[d,q] = V.T @ expT (accum over k)
      final[q,o] = outT.T @ WoT (accum over d); * 1/sums + bo; DMA out.

Softmax normalization commutes with the (linear) out-projection, so 1/sum is
applied on the final tiles where q sits on partitions.
"""

import numpy as np
import ml_dtypes

import concourse.bass as bass  # noqa: F401  (bass types used via bacc/tile)
import concourse.mybir as mybir
import concourse.tile as tile
from concourse import bacc
from concourse.bass_utils import run_bass_kernel_spmd

# ---- problem shapes (hardcoded) ----
B, SQ, SKV, D, C = 8, 2048, 2048, 1024, 768
P = 128
DT = D // P          # 8  d-tiles
CT = C // P          # 6  c-tiles
KT = SKV // P        # 16 k-tiles
QB = 512             # q block width
NQB = SQ // QB       # 4 q blocks
KC = 512             # k chunk width in phase 1a
NKC = SKV // KC      # 4
SCALE = 1.0 / np.sqrt(np.float32(D))
WARM_N = 18          # PE warmup matmuls covering the initial DMA window

F32 = mybir.dt.float32
F32R = mybir.dt.float32r
BF = mybir.dt.bfloat16
AF = mybir.ActivationFunctionType

_NC_CACHE = {}


def build():
    if "nc" in _NC_CACHE:
        return _NC_CACHE["nc"]
    nc = bacc.Bacc(trn_type="TRN2", num_swdge_queues=4)

    # ---- DRAM I/O (per-core slices; names = in_map keys) ----
    xT = nc.dram_tensor("xT", [D, SQ], BF, kind="ExternalInput")
    ctxT = nc.dram_tensor("ctxT", [C, SKV], BF, kind="ExternalInput")
    WqT = nc.dram_tensor("WqT", [D, D], BF, kind="ExternalInput")
    WkT = nc.dram_tensor("WkT", [C, D], BF, kind="ExternalInput")
    WvT = nc.dram_tensor("WvT", [C, D], BF, kind="ExternalInput")
    WoT = nc.dram_tensor("WoT", [D, D], BF, kind="ExternalInput")
    bqh = nc.dram_tensor("bqh", [P, DT], F32, kind="ExternalInput")  # bq*scale
    bkh = nc.dram_tensor("bkh", [P, DT], F32, kind="ExternalInput")
    bvb = nc.dram_tensor("bvb", [P, D], F32, kind="ExternalInput")   # bv bcast
    bob = nc.dram_tensor("bob", [P, D], F32, kind="ExternalInput")   # bo bcast
    onesmat = nc.dram_tensor("onesmat", [P, P], F32R, kind="ExternalInput")
    e0two = nc.dram_tensor("e0two", [P, 2], F32R, kind="ExternalInput")
    out = nc.dram_tensor("out", [SQ, D], F32, kind="ExternalOutput")

    with tile.TileContext(nc) as tc:
        with tc.tile_pool(name="persist", bufs=1) as persist:
            kt_sb = persist.tile([P, DT, SKV], BF, name="kt_sb")    # 32KB/p
            v_sb = persist.tile([P, KT, D], BF, name="v_sb")        # 32KB/p
            qt_sb = persist.tile([P, DT, SQ], BF, name="qt_sb")     # 32KB/p
            wo_sb = persist.tile([P, DT, D], BF, name="wo_sb")      # 16KB/p
            bq_sb = persist.tile([P, DT], F32, name="bq_sb")
            bk_sb = persist.tile([P, DT], F32, name="bk_sb")
            bv_sb = persist.tile([P, D], F32, name="bv_sb")
            bo_sb = persist.tile([P, D], F32, name="bo_sb")
            om_sb = persist.tile([P, P], F32R, name="om_sb")
            e0_sb = persist.tile([P, 2], F32R, name="e0_sb")
            warm_sb = persist.tile([P, 512], BF, name="warm_sb")
            sums_sb = persist.tile([P, QB], F32R, name="sums_sb")
            # consts go on the scalar HW queue, but AFTER ctx0 (emitted in
            # phase 1a below) — they're only needed at the first K evac

            # p1b pools opened early so x/wq prefetch DMAs can be issued
            # while phase 1a computes (SBUF: coexists with 1a pools)
            with tc.tile_pool(name="p1b_w", bufs=1) as p1b_w, \
                 tc.tile_pool(name="p1b_s", bufs=2) as p1b_s:
                wq_sb = p1b_w.tile([P, DT, D], BF, name="wq_sb")
                xt_tiles = [None] * NQB

                # ================= phase 1a: K^T + V resident =============
                with tc.tile_pool(name="p1a_w", bufs=1) as p1a_w, \
                     tc.tile_pool(name="p1a_s", bufs=2) as p1a_s, \
                     tc.tile_pool(name="ps_w", bufs=4, space="PSUM") as ps_w, \
                     tc.tile_pool(name="ps_k", bufs=2, space="PSUM") as ps_k, \
                     tc.tile_pool(name="ps_v", bufs=2, space="PSUM") as ps_v:
                    wk_sb = p1a_w.tile([P, CT, D], BF, name="wk_sb")
                    wv_sb = p1a_w.tile([P, CT, D], BF, name="wv_sb")

                    # PE warmup: keep the tensor engine busy (and its
                    # p-state at 2.4 GHz) while the first real operands
                    # stream in.  4 psum bufs so the pool-rotation
                    # semaphores resolve early and the warmups run
                    # back-to-back (2-buf rotation paced them at the
                    # ~420ns semaphore round-trip, which kept resetting
                    # the p-state ramp).
                    nc.gpsimd.memset(warm_sb[:], 0.0)
                    for _ in range(WARM_N):
                        pw = ps_w.tile([P, 512], F32, name="pw", tag="pw")
                        nc.tensor.matmul(pw, warm_sb[:, 0:P], warm_sb[:],
                                         start=True, stop=True)

                    # gating loads split across both HW queues so the first
                    # K group's operands land in parallel:
                    #   sync:   wk_h0, wk_h1, ctx1, ...
                    #   scalar: ctx0, consts, wv_h0, wv_h1, ...
                    nc.sync.dma_start(
                        wk_sb[:, :, 0:512],
                        WkT[:, 0:512].rearrange("(c p) d -> p c d", p=P))
                    ctx_tiles = [None] * NKC
                    for kc in range(2):
                        ctx_tiles[kc] = p1a_s.tile([P, CT, KC], BF,
                                                   name="ctx_sb", tag="ctx")
                    nc.scalar.dma_start(
                        ctx_tiles[0][:],
                        ctxT[:, 0:KC].rearrange("(c p) k -> p c k", p=P))
                    # consts on the (otherwise idle) gpsimd SW queue so
                    # their descriptor writes don't delay the scalar
                    # engine's time-critical K evacuations
                    nc.gpsimd.dma_start(bq_sb, bqh[:])
                    nc.gpsimd.dma_start(bk_sb, bkh[:])
                    nc.gpsimd.dma_start(bv_sb, bvb[:])
                    nc.gpsimd.dma_start(bo_sb, bob[:])
                    nc.gpsimd.dma_start(om_sb, onesmat[:])
                    nc.gpsimd.dma_start(e0_sb, e0two[:])
                    nc.sync.dma_start(
                        wk_sb[:, :, 512:1024],
                        WkT[:, 512:1024].rearrange("(c p) d -> p c d", p=P))
                    nc.sync.dma_start(
                        ctx_tiles[1][:],
                        ctxT[:, KC:2 * KC].rearrange("(c p) k -> p c k", p=P))
                    # scalar queue: wv halves (V needed ~10us in)
                    for h in range(2):
                        nc.scalar.dma_start(
                            wv_sb[:, :, h * 512:(h + 1) * 512],
                            WvT[:, h * 512:(h + 1) * 512].rearrange(
                                "(c p) d -> p c d", p=P))

                    for kc in range(NKC):
                        if ctx_tiles[kc] is None:
                            ctx_tiles[kc] = p1a_s.tile([P, CT, KC], BF,
                                                       name="ctx_sb",
                                                       tag="ctx")
                            nc.sync.dma_start(
                                ctx_tiles[kc][:],
                                ctxT[:, kc * KC:(kc + 1) * KC].rearrange(
                                    "(c p) k -> p c k", p=P))
                        ctx_sb = ctx_tiles[kc]
                        # K^T tiles [d=128, k=512] per d-tile
                        for dt_ in range(DT):
                            pk = ps_k.tile([P, KC], F32, name="pk", tag="pk")
                            for ct_ in range(CT):
                                nc.tensor.matmul(
                                    pk, wk_sb[:, ct_, dt_ * P:(dt_ + 1) * P],
                                    ctx_sb[:, ct_, :],
                                    start=(ct_ == 0), stop=(ct_ == CT - 1))
                            nc.scalar.activation(
                                kt_sb[:, dt_, kc * KC:(kc + 1) * KC], pk,
                                AF.Identity, bias=bk_sb[:, dt_:dt_ + 1])
                        # V tiles [k=128, d] resident
                        for t in range(4):
                            kt_ = kc * 4 + t
                            for dh in range(2):
                                pv = ps_v.tile([P, 512], F32, name="pv",
                                               tag="pv")
                                for ct_ in range(CT):
                                    nc.tensor.matmul(
                                        pv, ctx_sb[:, ct_, t * P:(t + 1) * P],
                                        wv_sb[:, ct_, dh * 512:(dh + 1) * 512],
                                        start=(ct_ == 0), stop=(ct_ == CT - 1))
                                nc.vector.tensor_add(
                                    v_sb[:, kt_, dh * 512:(dh + 1) * 512],
                                    pv, bv_sb[:, dh * 512:(dh + 1) * 512])
                        # prefetches interleaved into the queues mid-1a
                        if kc == 0:
                            nc.scalar.dma_start(
                                wq_sb[:, :, 0:512],
                                WqT[:, 0:512].rearrange(
                                    "(i p) d -> p i d", p=P))
                        if kc == 1:
                            ctx_tiles[2] = p1a_s.tile([P, CT, KC], BF,
                                                      name="ctx_sb",
                                                      tag="ctx")
                            nc.sync.dma_start(
                                ctx_tiles[2][:],
                                ctxT[:, 2 * KC:3 * KC].rearrange(
                                    "(c p) k -> p c k", p=P))
                            nc.scalar.dma_start(
                                wq_sb[:, :, 512:1024],
                                WqT[:, 512:1024].rearrange(
                                    "(i p) d -> p i d", p=P))
                            xt_tiles[0] = p1b_s.tile([P, DT, QB], BF,
                                                     name="xt_sb", tag="xt")
                            nc.sync.dma_start(
                                xt_tiles[0][:],
                                xT[:, 0:QB].rearrange("(i p) q -> p i q", p=P))
                        if kc == 2:
                            ctx_tiles[3] = p1a_s.tile([P, CT, KC], BF,
                                                      name="ctx_sb",
                                                      tag="ctx")
                            nc.sync.dma_start(
                                ctx_tiles[3][:],
                                ctxT[:, 3 * KC:4 * KC].rearrange(
                                    "(c p) k -> p c k", p=P))
                            xt_tiles[1] = p1b_s.tile([P, DT, QB], BF,
                                                     name="xt_sb", tag="xt")
                            nc.sync.dma_start(
                                xt_tiles[1][:],
                                xT[:, QB:2 * QB].rearrange(
                                    "(i p) q -> p i q", p=P))
                            nc.scalar.dma_start(
                                wo_sb[:, :, 0:512],
                                WoT[:, 0:512].rearrange(
                                    "(i p) d -> p i d", p=P))
                        if kc == 3:
                            nc.scalar.dma_start(
                                wo_sb[:, :, 512:1024],
                                WoT[:, 512:1024].rearrange(
                                    "(i p) d -> p i d", p=P))

                # ================= phase 1b: Q^T resident =================
                with tc.tile_pool(name="ps_q", bufs=2, space="PSUM") as ps_q:
                    for qb in range(NQB):
                        if xt_tiles[qb] is None:
                            xt_tiles[qb] = p1b_s.tile([P, DT, QB], BF,
                                                      name="xt_sb", tag="xt")
                            nc.sync.dma_start(
                                xt_tiles[qb][:],
                                xT[:, qb * QB:(qb + 1) * QB].rearrange(
                                    "(i p) q -> p i q", p=P))
                        xt_sb = xt_tiles[qb]
                        for dt_ in range(DT):
                            pq = ps_q.tile([P, QB], F32, name="pq", tag="pq")
                            for it in range(DT):
                                nc.tensor.matmul(
                                    pq, wq_sb[:, it, dt_ * P:(dt_ + 1) * P],
                                    xt_sb[:, it, :],
                                    start=(it == 0), stop=(it == DT - 1))
                            nc.scalar.activation(
                                qt_sb[:, dt_, qb * QB:(qb + 1) * QB], pq,
                                AF.Identity, bias=bq_sb[:, dt_:dt_ + 1],
                                scale=float(SCALE))

            # ============== phase 2: attention + out proj =============
            with tc.tile_pool(name="p2_big", bufs=1) as p2_big, \
                 tc.tile_pool(name="p2_par", bufs=2) as p2_par, \
                 tc.tile_pool(name="p2_fin", bufs=4) as p2_fin, \
                 tc.tile_pool(name="p2_rcp", bufs=2) as p2_rcp, \
                 tc.tile_pool(name="ps_sc", bufs=2, space="PSUM") as ps_sc, \
                 tc.tile_pool(name="ps_sum", bufs=1, space="PSUM") as ps_sum, \
                 tc.tile_pool(name="ps_rt", bufs=1, space="PSUM") as ps_rt, \
                 tc.tile_pool(name="ps_out", bufs=2, space="PSUM") as ps_out, \
                 tc.tile_pool(name="ps_fin", bufs=2, space="PSUM") as ps_fin:
              for qb in range(NQB):
                expt_sb = p2_big.tile([P, KT, QB], BF, name="expt_sb",
                                      tag="expt")
                partial = p2_par.tile([P, QB], F32R, name="partial",
                                      tag="par")
                # ---- scores^T + exp + DVE partial-sum chain ----
                for kt_ in range(KT):
                    psc = ps_sc.tile([P, QB], F32, name="psc", tag="psc")
                    for dt_ in range(DT):
                        nc.tensor.matmul(
                            psc, kt_sb[:, dt_, kt_ * P:(kt_ + 1) * P],
                            qt_sb[:, dt_, qb * QB:(qb + 1) * QB],
                            start=(dt_ == 0), stop=(dt_ == DT - 1))
                    nc.scalar.activation(expt_sb[:, kt_], psc, AF.Exp)
                    if kt_ == 0:
                        nc.vector.tensor_copy(partial, expt_sb[:, 0])
                    else:
                        nc.vector.tensor_add(partial, partial,
                                             expt_sb[:, kt_])
                # ---- out^T = V.T @ expT (d-quarter passes) ----
                outt_sb = p2_big.tile([P, DT, QB], BF, name="outt_sb",
                                      tag="outt")
                for dp in range(4):
                    po0 = ps_out.tile([P, QB], F32, name="po0", tag="po")
                    po1 = ps_out.tile([P, QB], F32, name="po1", tag="po")
                    po = (po0, po1)
                    for kt_ in range(KT):
                        for dc in range(2):
                            d0 = dp * 256 + dc * P
                            nc.tensor.matmul(
                                po[dc], v_sb[:, kt_, d0:d0 + P],
                                expt_sb[:, kt_],
                                start=(kt_ == 0), stop=(kt_ == KT - 1))
                    for dc in range(2):
                        nc.scalar.copy(outt_sb[:, dp * 2 + dc], po[dc])
                # ---- sums over partitions (1 matmul) + 1/sums on q ----
                # (emitted after PV so the exp[15] -> DVE-chain latency
                # hides under the PV matmuls instead of stalling the PE)
                psums = ps_sum.tile([P, QB], F32, name="psums", tag="psums")
                nc.tensor.matmul(psums, om_sb, partial, start=True, stop=True)
                nc.scalar.copy(sums_sb, psums)
                prt = ps_rt.tile([P, 8], F32, name="prt", tag="prt")
                for qs in range(4):
                    nc.tensor.matmul(
                        prt[:, 2 * qs:2 * qs + 2],
                        sums_sb[:, qs * P:(qs + 1) * P], e0_sb,
                        start=True, stop=True)
                recip = p2_rcp.tile([P, 8], F32, name="recip", tag="recip")
                nc.vector.reciprocal(recip, prt)
                # ---- final = out^T.T @ WoT, * 1/sums + bo ----
                for qs in range(4):
                    for oc in range(2):
                        pf = ps_fin.tile([P, 512], F32, name="pf", tag="pf")
                        for dt_ in range(DT):
                            nc.tensor.matmul(
                                pf, outt_sb[:, dt_, qs * P:(qs + 1) * P],
                                wo_sb[:, dt_, oc * 512:(oc + 1) * 512],
                                start=(dt_ == 0), stop=(dt_ == DT - 1))
                        fin = p2_fin.tile([P, 512], F32, name="fin",
                                          tag="fin")
                        nc.scalar.mul(fin, pf, recip[:, 2 * qs:2 * qs + 1])
                        nc.vector.tensor_add(
                            fin, fin, bo_sb[:, oc * 512:(oc + 1) * 512])
                        # outputs on the sync HW queue (idle in phase 2;
                        # the gpsimd SW queue made the final write the
                        # kernel's tail)
                        nc.sync.dma_start(
                            out[qb * QB + qs * P: qb * QB + (qs + 1) * P,
                                oc * 512:(oc + 1) * 512], fin)
    nc.finalize()
    _NC_CACHE["nc"] = nc
    return nc


def _host_prep(x, context, Wq, bq, Wk, bk, Wv, bv, Wo, bo):
    """Build the 8 per-core input maps (host-side layout prep)."""
    bf = ml_dtypes.bfloat16
    x = np.asarray(x, dtype=np.float32)
    context = np.asarray(context, dtype=np.float32)
    WqT = np.ascontiguousarray(np.asarray(Wq, np.float32).T).astype(bf)
    WkT = np.ascontiguousarray(np.asarray(Wk, np.float32).T).astype(bf)
    WvT = np.ascontiguousarray(np.asarray(Wv, np.float32).T).astype(bf)
    WoT = np.ascontiguousarray(np.asarray(Wo, np.float32).T).astype(bf)
    scale = np.float32(1.0 / np.sqrt(np.float32(D)))
    bqh = np.ascontiguousarray(
        (np.asarray(bq, np.float32) * scale).reshape(DT, P).T)  # [p, dt]
    bkh = np.ascontiguousarray(np.asarray(bk, np.float32).reshape(DT, P).T)
    bvb = np.ascontiguousarray(
        np.broadcast_to(np.asarray(bv, np.float32)[None, :], (P, D)))
    bob = np.ascontiguousarray(
        np.broadcast_to(np.asarray(bo, np.float32)[None, :], (P, D)))
    onesmat = np.ones((P, P), np.float32)
    e0two = np.zeros((P, 2), np.float32)
    e0two[0, :] = 1.0
    shared = dict(WqT=WqT, WkT=WkT, WvT=WvT, WoT=WoT, bqh=bqh, bkh=bkh,
                  bvb=bvb, bob=bob, onesmat=onesmat, e0two=e0two)
    in_maps = []
    for b in range(B):
        m = dict(shared)
        m["xT"] = np.ascontiguousarray(x[b].T).astype(bf)        # [D, SQ]
        m["ctxT"] = np.ascontiguousarray(context[b].T).astype(bf)  # [C, SKV]
        in_maps.append(m)
    return in_maps


def kernel(**inputs) -> np.ndarray:
    nc = build()
    in_maps = _host_prep(**inputs)
    res = run_bass_kernel_spmd(nc, in_maps, core_ids=list(range(B)))
    return np.stack([res.results[b]["out"] for b in range(B)], axis=0)


# revision 15
# speedup vs baseline: 1.2989x; 1.0018x over previous
"""Trainium2 Bass kernel for nn_CrossAttention (B=8, Sq=Skv=2048, D=1024, C=768).

Strategy: data-parallel over batch — each of the 8 NeuronCores computes one
batch element's full cross-attention.

v2 design (vs v1 which staged K^T/Q^T via DRAM in f32r):
  * all matmul operands in bf16 — same PE throughput as f32r at free>=256
    (1 cycle/row) but half the DMA traffic and half the SBUF footprint.
  * K^T, V, Q^T and Wo live fully SBUF-resident; zero intermediate DRAM
    staging (v1 round-tripped 32 MB through HBM).
  * inputs stream in over TWO hardware DGE queues (sync + scalar) in
    compute-need order as a handful of large rearranged DMAs.
  * PE warmup matmuls at t=0 keep the tensor engine's p-state at full clock
    while the first operands land (idle PE restarts at 1.2 GHz for ~3 us).
  * softmax denominators accumulated across k-tiles on the DVE (16 tensor_adds)
    + one ones-matmul per q-block instead of 16 PE matmuls per q-block.

Per-core pipeline:
  phase 1a: K^T[d,k] = Wk @ ctx^T (+bk) -> SBUF bf16; V[k,d] = ctx @ Wv^T
            (+bv) -> SBUF bf16.  (per 512-wide k chunk)
  phase 1b: Q^T[d,q] = (Wq @ x^T + bq)/sqrt(D) -> SBUF bf16. (per 512 q chunk)
  phase 2 (per 512-wide q block):
      scores^T[k,q] = KT.T @ QT (accum over d)           -> PSUM
      expT = exp(scores^T) (ACT evac, no max: |s| small)  -> SBUF bf16
      partial[p,q] += expT[kt]  on DVE;  sums = ones.T @ partial (1 matmul)
      out^T[d,q] = V.T @ expT (accum over k)
      final[q,o] = outT.T @ WoT (accum over d); * 1/sums + bo; DMA out.

Softmax normalization commutes with the (linear) out-projection, so 1/sum is
applied on the final tiles where q sits on partitions.
"""

import numpy as np
import ml_dtypes

import concourse.bass as bass  # noqa: F401  (bass types used via bacc/tile)
import concourse.mybir as mybir
import concourse.tile as tile
from concourse import bacc
from concourse.bass_utils import run_bass_kernel_spmd

# ---- problem shapes (hardcoded) ----
B, SQ, SKV, D, C = 8, 2048, 2048, 1024, 768
P = 128
DT = D // P          # 8  d-tiles
CT = C // P          # 6  c-tiles
KT = SKV // P        # 16 k-tiles
QB = 512             # q block width
NQB = SQ // QB       # 4 q blocks
KC = 512             # k chunk width in phase 1a
NKC = SKV // KC      # 4
SCALE = 1.0 / np.sqrt(np.float32(D))
WARM_N = 15          # PE warmup matmuls covering the initial DMA window

F32 = mybir.dt.float32
F32R = mybir.dt.float32r
BF = mybir.dt.bfloat16
AF = mybir.ActivationFunctionType

_NC_CACHE = {}


def build():
    if "nc" in _NC_CACHE:
        return _NC_CACHE["nc"]
    nc = bacc.Bacc(trn_type="TRN2", num_swdge_queues=4)

    # ---- DRAM I/O (per-core slices; names = in_map keys) ----
    xT = nc.dram_tensor("xT", [D, SQ], BF, kind="ExternalInput")
    ctxT = nc.dram_tensor("ctxT", [C, SKV], BF, kind="ExternalInput")
    WqT = nc.dram_tensor("WqT", [D, D], BF, kind="ExternalInput")
    WkT = nc.dram_tensor("WkT", [C, D], BF, kind="ExternalInput")
    WvT = nc.dram_tensor("WvT", [C, D], BF, kind="ExternalInput")
    WoT = nc.dram_tensor("WoT", [D, D], BF, kind="ExternalInput")
    bqh = nc.dram_tensor("bqh", [P, DT], F32, kind="ExternalInput")  # bq*scale
    bkh = nc.dram_tensor("bkh", [P, DT], F32, kind="ExternalInput")
    bvb = nc.dram_tensor("bvb", [P, D], F32, kind="ExternalInput")   # bv bcast
    bob = nc.dram_tensor("bob", [P, D], F32, kind="ExternalInput")   # bo bcast
    onesmat = nc.dram_tensor("onesmat", [P, P], F32R, kind="ExternalInput")
    e0two = nc.dram_tensor("e0two", [P, 2], F32R, kind="ExternalInput")
    out = nc.dram_tensor("out", [SQ, D], F32, kind="ExternalOutput")

    with tile.TileContext(nc) as tc:
        with tc.tile_pool(name="persist", bufs=1) as persist:
            kt_sb = persist.tile([P, DT, SKV], BF, name="kt_sb")    # 32KB/p
            v_sb = persist.tile([P, KT, D], BF, name="v_sb")        # 32KB/p
            qt_sb = persist.tile([P, DT, SQ], BF, name="qt_sb")     # 32KB/p
            wo_sb = persist.tile([P, DT, D], BF, name="wo_sb")      # 16KB/p
            bq_sb = persist.tile([P, DT], F32, name="bq_sb")
            bk_sb = persist.tile([P, DT], F32, name="bk_sb")
            bv_sb = persist.tile([P, D], F32, name="bv_sb")
            bo_sb = persist.tile([P, D], F32, name="bo_sb")
            om_sb = persist.tile([P, P], F32R, name="om_sb")
            e0_sb = persist.tile([P, 2], F32R, name="e0_sb")
            warm_sb = persist.tile([P, 512], BF, name="warm_sb")
            sums_sb = persist.tile([P, QB], F32R, name="sums_sb")
            # consts go on the scalar HW queue, but AFTER ctx0 (emitted in
            # phase 1a below) — they're only needed at the first K evac

            # p1b pools opened early so x/wq prefetch DMAs can be issued
            # while phase 1a computes (SBUF: coexists with 1a pools)
            with tc.tile_pool(name="p1b_w", bufs=1) as p1b_w, \
                 tc.tile_pool(name="p1b_s", bufs=2) as p1b_s:
                wq_sb = p1b_w.tile([P, DT, D], BF, name="wq_sb")
                xt_tiles = [None] * NQB

                # ================= phase 1a: K^T + V resident =============
                with tc.tile_pool(name="p1a_w", bufs=1) as p1a_w, \
                     tc.tile_pool(name="p1a_s", bufs=2) as p1a_s, \
                     tc.tile_pool(name="ps_w", bufs=4, space="PSUM") as ps_w, \
                     tc.tile_pool(name="ps_k", bufs=2, space="PSUM") as ps_k, \
                     tc.tile_pool(name="ps_v", bufs=2, space="PSUM") as ps_v:
                    wk_sb = p1a_w.tile([P, CT, D], BF, name="wk_sb")
                    wv_sb = p1a_w.tile([P, CT, D], BF, name="wv_sb")

                    # PE warmup: keep the tensor engine busy (and its
                    # p-state at 2.4 GHz) while the first real operands
                    # stream in.  4 psum bufs so the pool-rotation
                    # semaphores resolve early and the warmups run
                    # back-to-back (2-buf rotation paced them at the
                    # ~420ns semaphore round-trip, which kept resetting
                    # the p-state ramp).
                    nc.gpsimd.memset(warm_sb[:], 0.0)
                    for _ in range(WARM_N):
                        pw = ps_w.tile([P, 512], F32, name="pw", tag="pw")
                        nc.tensor.matmul(pw, warm_sb[:, 0:P], warm_sb[:],
                                         start=True, stop=True)

                    # The two HW queues SHARE ~350 GB/s of HBM bandwidth, so
                    # global need-order matters more than queue parallelism:
                    # nothing but the gating pair (wk_h0 + ctx0) may be in
                    # flight until the first K group can start.
                    #   sync:   wk_h0, wk_h1, bvb, ctx1, ...
                    #   scalar: ctx0, bkh, wv_h0, wv_h1, ...
                    nc.sync.dma_start(
                        wk_sb[:, :, 0:512],
                        WkT[:, 0:512].rearrange("(c p) d -> p c d", p=P))
                    ctx_tiles = [None] * NKC
                    for kc in range(2):
                        ctx_tiles[kc] = p1a_s.tile([P, CT, KC], BF,
                                                   name="ctx_sb", tag="ctx")
                    nc.scalar.dma_start(
                        ctx_tiles[0][:],
                        ctxT[:, 0:KC].rearrange("(c p) k -> p c k", p=P))
                    nc.sync.dma_start(
                        wk_sb[:, :, 512:1024],
                        WkT[:, 512:1024].rearrange("(c p) d -> p c d", p=P))
                    nc.scalar.dma_start(bk_sb, bkh[:])
                    for h in range(2):
                        nc.scalar.dma_start(
                            wv_sb[:, :, h * 512:(h + 1) * 512],
                            WvT[:, h * 512:(h + 1) * 512].rearrange(
                                "(c p) d -> p c d", p=P))
                    nc.sync.dma_start(bv_sb, bvb[:])
                    nc.sync.dma_start(
                        ctx_tiles[1][:],
                        ctxT[:, KC:2 * KC].rearrange("(c p) k -> p c k", p=P))

                    for kc in range(NKC):
                        if ctx_tiles[kc] is None:
                            ctx_tiles[kc] = p1a_s.tile([P, CT, KC], BF,
                                                       name="ctx_sb",
                                                       tag="ctx")
                            nc.sync.dma_start(
                                ctx_tiles[kc][:],
                                ctxT[:, kc * KC:(kc + 1) * KC].rearrange(
                                    "(c p) k -> p c k", p=P))
                        ctx_sb = ctx_tiles[kc]
                        # K^T tiles [d=128, k=512] per d-tile
                        for dt_ in range(DT):
                            pk = ps_k.tile([P, KC], F32, name="pk", tag="pk")
                            for ct_ in range(CT):
                                nc.tensor.matmul(
                                    pk, wk_sb[:, ct_, dt_ * P:(dt_ + 1) * P],
                                    ctx_sb[:, ct_, :],
                                    start=(ct_ == 0), stop=(ct_ == CT - 1))
                            nc.scalar.activation(
                                kt_sb[:, dt_, kc * KC:(kc + 1) * KC], pk,
                                AF.Identity, bias=bk_sb[:, dt_:dt_ + 1])
                        # V tiles [k=128, d] resident
                        for t in range(4):
                            kt_ = kc * 4 + t
                            for dh in range(2):
                                pv = ps_v.tile([P, 512], F32, name="pv",
                                               tag="pv")
                                for ct_ in range(CT):
                                    nc.tensor.matmul(
                                        pv, ctx_sb[:, ct_, t * P:(t + 1) * P],
                                        wv_sb[:, ct_, dh * 512:(dh + 1) * 512],
                                        start=(ct_ == 0), stop=(ct_ == CT - 1))
                                nc.vector.tensor_add(
                                    v_sb[:, kt_, dh * 512:(dh + 1) * 512],
                                    pv, bv_sb[:, dh * 512:(dh + 1) * 512])
                        # prefetches interleaved into the queues mid-1a
                        if kc == 0:
                            nc.scalar.dma_start(
                                wq_sb[:, :, 0:512],
                                WqT[:, 0:512].rearrange(
                                    "(i p) d -> p i d", p=P))
                            nc.scalar.dma_start(bq_sb, bqh[:])
                        if kc == 1:
                            ctx_tiles[2] = p1a_s.tile([P, CT, KC], BF,
                                                      name="ctx_sb",
                                                      tag="ctx")
                            nc.sync.dma_start(
                                ctx_tiles[2][:],
                                ctxT[:, 2 * KC:3 * KC].rearrange(
                                    "(c p) k -> p c k", p=P))
                            nc.scalar.dma_start(
                                wq_sb[:, :, 512:1024],
                                WqT[:, 512:1024].rearrange(
                                    "(i p) d -> p i d", p=P))
                            nc.scalar.dma_start(om_sb, onesmat[:])
                            nc.scalar.dma_start(e0_sb, e0two[:])
                            xt_tiles[0] = p1b_s.tile([P, DT, QB], BF,
                                                     name="xt_sb", tag="xt")
                            nc.sync.dma_start(
                                xt_tiles[0][:],
                                xT[:, 0:QB].rearrange("(i p) q -> p i q", p=P))
                        if kc == 2:
                            ctx_tiles[3] = p1a_s.tile([P, CT, KC], BF,
                                                      name="ctx_sb",
                                                      tag="ctx")
                            nc.sync.dma_start(
                                ctx_tiles[3][:],
                                ctxT[:, 3 * KC:4 * KC].rearrange(
                                    "(c p) k -> p c k", p=P))
                            xt_tiles[1] = p1b_s.tile([P, DT, QB], BF,
                                                     name="xt_sb", tag="xt")
                            nc.sync.dma_start(
                                xt_tiles[1][:],
                                xT[:, QB:2 * QB].rearrange(
                                    "(i p) q -> p i q", p=P))
                            nc.scalar.dma_start(
                                wo_sb[:, :, 0:512],
                                WoT[:, 0:512].rearrange(
                                    "(i p) d -> p i d", p=P))
                            nc.scalar.dma_start(bo_sb, bob[:])
                        if kc == 3:
                            nc.scalar.dma_start(
                                wo_sb[:, :, 512:1024],
                                WoT[:, 512:1024].rearrange(
                                    "(i p) d -> p i d", p=P))

                # ================= phase 1b: Q^T resident =================
                with tc.tile_pool(name="ps_q", bufs=2, space="PSUM") as ps_q:
                    for qb in range(NQB):
                        if xt_tiles[qb] is None:
                            xt_tiles[qb] = p1b_s.tile([P, DT, QB], BF,
                                                      name="xt_sb", tag="xt")
                            nc.sync.dma_start(
                                xt_tiles[qb][:],
                                xT[:, qb * QB:(qb + 1) * QB].rearrange(
                                    "(i p) q -> p i q", p=P))
                        xt_sb = xt_tiles[qb]
                        for dt_ in range(DT):
                            pq = ps_q.tile([P, QB], F32, name="pq", tag="pq")
                            for it in range(DT):
                                nc.tensor.matmul(
                                    pq, wq_sb[:, it, dt_ * P:(dt_ + 1) * P],
                                    xt_sb[:, it, :],
                                    start=(it == 0), stop=(it == DT - 1))
                            nc.scalar.activation(
                                qt_sb[:, dt_, qb * QB:(qb + 1) * QB], pq,
                                AF.Identity, bias=bq_sb[:, dt_:dt_ + 1],
                                scale=float(SCALE))

            # ============== phase 2: attention + out proj =============
            with tc.tile_pool(name="p2_big", bufs=1) as p2_big, \
                 tc.tile_pool(name="p2_par", bufs=2) as p2_par, \
                 tc.tile_pool(name="p2_fin", bufs=4) as p2_fin, \
                 tc.tile_pool(name="p2_rcp", bufs=2) as p2_rcp, \
                 tc.tile_pool(name="ps_sc", bufs=2, space="PSUM") as ps_sc, \
                 tc.tile_pool(name="ps_sum", bufs=1, space="PSUM") as ps_sum, \
                 tc.tile_pool(name="ps_rt", bufs=1, space="PSUM") as ps_rt, \
                 tc.tile_pool(name="ps_out", bufs=2, space="PSUM") as ps_out, \
                 tc.tile_pool(name="ps_fin", bufs=2, space="PSUM") as ps_fin:
              for qb in range(NQB):
                expt_sb = p2_big.tile([P, KT, QB], BF, name="expt_sb",
                                      tag="expt")
                partial = p2_par.tile([P, QB], F32R, name="partial",
                                      tag="par")
                # ---- scores^T + exp + DVE partial-sum chain ----
                for kt_ in range(KT):
                    psc = ps_sc.tile([P, QB], F32, name="psc", tag="psc")
                    for dt_ in range(DT):
                        nc.tensor.matmul(
                            psc, kt_sb[:, dt_, kt_ * P:(kt_ + 1) * P],
                            qt_sb[:, dt_, qb * QB:(qb + 1) * QB],
                            start=(dt_ == 0), stop=(dt_ == DT - 1))
                    nc.scalar.activation(expt_sb[:, kt_], psc, AF.Exp)
                    if kt_ == 0:
                        nc.vector.tensor_copy(partial, expt_sb[:, 0])
                    else:
                        nc.vector.tensor_add(partial, partial,
                                             expt_sb[:, kt_])
                # ---- out^T = V.T @ expT (d-quarter passes) ----
                outt_sb = p2_big.tile([P, DT, QB], BF, name="outt_sb",
                                      tag="outt")
                for dp in range(4):
                    po0 = ps_out.tile([P, QB], F32, name="po0", tag="po")
                    po1 = ps_out.tile([P, QB], F32, name="po1", tag="po")
                    po = (po0, po1)
                    for kt_ in range(KT):
                        for dc in range(2):
                            d0 = dp * 256 + dc * P
                            nc.tensor.matmul(
                                po[dc], v_sb[:, kt_, d0:d0 + P],
                                expt_sb[:, kt_],
                                start=(kt_ == 0), stop=(kt_ == KT - 1))
                    for dc in range(2):
                        nc.scalar.copy(outt_sb[:, dp * 2 + dc], po[dc])
                # ---- sums over partitions (1 matmul) + 1/sums on q ----
                # (emitted after PV so the exp[15] -> DVE-chain latency
                # hides under the PV matmuls instead of stalling the PE)
                psums = ps_sum.tile([P, QB], F32, name="psums", tag="psums")
                nc.tensor.matmul(psums, om_sb, partial, start=True, stop=True)
                nc.scalar.copy(sums_sb, psums)
                prt = ps_rt.tile([P, 8], F32, name="prt", tag="prt")
                for qs in range(4):
                    nc.tensor.matmul(
                        prt[:, 2 * qs:2 * qs + 2],
                        sums_sb[:, qs * P:(qs + 1) * P], e0_sb,
                        start=True, stop=True)
                recip = p2_rcp.tile([P, 8], F32, name="recip", tag="recip")
                nc.vector.reciprocal(recip, prt)
                # ---- final = out^T.T @ WoT, * 1/sums + bo ----
                for qs in range(4):
                    for oc in range(2):
                        pf = ps_fin.tile([P, 512], F32, name="pf", tag="pf")
                        for dt_ in range(DT):
                            nc.tensor.matmul(
                                pf, outt_sb[:, dt_, qs * P:(qs + 1) * P],
                                wo_sb[:, dt_, oc * 512:(oc + 1) * 512],
                                start=(dt_ == 0), stop=(dt_ == DT - 1))
                        fin = p2_fin.tile([P, 512], F32, name="fin",
                                          tag="fin")
                        nc.scalar.mul(fin, pf, recip[:, 2 * qs:2 * qs + 1])
                        nc.vector.tensor_add(
                            fin, fin, bo_sb[:, oc * 512:(oc + 1) * 512])
                        # outputs on the sync HW queue (idle in phase 2;
                        # the gpsimd SW queue made the final write the
                        # kernel's tail)
                        nc.sync.dma_start(
                            out[qb * QB + qs * P: qb * QB + (qs + 1) * P,
                                oc * 512:(oc + 1) * 512], fin)
    nc.finalize()
    _NC_CACHE["nc"] = nc
    return nc


def _host_prep(x, context, Wq, bq, Wk, bk, Wv, bv, Wo, bo):
    """Build the 8 per-core input maps (host-side layout prep)."""
    bf = ml_dtypes.bfloat16
    x = np.asarray(x, dtype=np.float32)
    context = np.asarray(context, dtype=np.float32)
    WqT = np.ascontiguousarray(np.asarray(Wq, np.float32).T).astype(bf)
    WkT = np.ascontiguousarray(np.asarray(Wk, np.float32).T).astype(bf)
    WvT = np.ascontiguousarray(np.asarray(Wv, np.float32).T).astype(bf)
    WoT = np.ascontiguousarray(np.asarray(Wo, np.float32).T).astype(bf)
    scale = np.float32(1.0 / np.sqrt(np.float32(D)))
    bqh = np.ascontiguousarray(
        (np.asarray(bq, np.float32) * scale).reshape(DT, P).T)  # [p, dt]
    bkh = np.ascontiguousarray(np.asarray(bk, np.float32).reshape(DT, P).T)
    bvb = np.ascontiguousarray(
        np.broadcast_to(np.asarray(bv, np.float32)[None, :], (P, D)))
    bob = np.ascontiguousarray(
        np.broadcast_to(np.asarray(bo, np.float32)[None, :], (P, D)))
    onesmat = np.ones((P, P), np.float32)
    e0two = np.zeros((P, 2), np.float32)
    e0two[0, :] = 1.0
    shared = dict(WqT=WqT, WkT=WkT, WvT=WvT, WoT=WoT, bqh=bqh, bkh=bkh,
                  bvb=bvb, bob=bob, onesmat=onesmat, e0two=e0two)
    in_maps = []
    for b in range(B):
        m = dict(shared)
        m["xT"] = np.ascontiguousarray(x[b].T).astype(bf)        # [D, SQ]
        m["ctxT"] = np.ascontiguousarray(context[b].T).astype(bf)  # [C, SKV]
        in_maps.append(m)
    return in_maps


def kernel(**inputs) -> np.ndarray:
    nc = build()
    in_maps = _host_prep(**inputs)
    res = run_bass_kernel_spmd(nc, in_maps, core_ids=list(range(B)))
    return np.stack([res.results[b]["out"] for b in range(B)], axis=0)
